# revision 8
# baseline (speedup 1.0000x reference)
"""Trainium2 Bass kernel for: out = conv3x3(x, weight*A_w) * sigmoid(conv3x3(relu(conv3x3(x, se_w1)), se_w2))

Sharding: data-parallel over batch B=8 -> 8 NeuronCores (one image per core);
weight / A_w / se_w1 / se_w2 replicated to every core.

Per-core kernel (direct conv as implicit GEMM on the TensorEngine):
  - x stored column-padded [ci, 56, 58] in SBUF (pad cols zeroed, +1-element
    guards at both flat ends) so every 3x3 tap is a contiguous 1-D shifted
    window (the matmul ISA requires single-free-dim operands).
  - row taps at the image top/bottom use clipped row ranges; with the center
    tap issued first (full coverage, start=True) the clipped taps accumulate,
    which implements zero padding exactly.
  - weights transposed on-device (PE transpose) to [ci, co] per tap; A_w is
    applied as a ScalarE per-partition scale during the PSUM->SBUF copy.
  - float32r matmul dtype: full-rate fp32 on the PE at free-dim >= 256
    (measured rel-err vs fp32 ~1.5e-4).
  - junk values only ever land in pad columns; they stay finite (sigmoid etc.)
    and are dropped by the strided output DMA.
"""

import numpy as np

import concourse.bass as bass  # noqa: F401
import concourse.mybir as mybir
import concourse.tile as tile
from concourse import bacc
from concourse.bass_utils import run_bass_kernel_spmd
from concourse.masks import make_identity

B, C, H, W = 8, 256, 56, 56
WP = W + 2                      # padded row width (c=0 left pad, c=57 right pad)
HWP = H * WP                    # 3248
CMID = 16
N_CORES = 8
RT = 8                          # output rows per PSUM tile
NT = H // RT                    # 7
TF = RT * WP                    # 464 floats per full PSUM tile (<=512, >=256)
F32 = mybir.dt.float32
F32R = mybir.dt.float32r

TAPS = [(0, 0)] + [
    (dh, dw) for dh in (-1, 0, 1) for dw in (-1, 0, 1) if (dh, dw) != (0, 0)
]


def _rows(r0, dh):
    """Clipped local row range [rl, rh) of a tile at base row r0 for row-tap dh."""
    return max(0, -dh - r0), min(RT, H - dh - r0)


def build():
    nc = bacc.Bacc("TRN2", target_bir_lowering=False, debug=False, num_devices=N_CORES)

    x_d = nc.dram_tensor("x", [C, H, W], F32, kind="ExternalInput").ap()
    w_d = nc.dram_tensor("weight", [C, C, 3, 3], F32, kind="ExternalInput").ap()
    aw_d = nc.dram_tensor("A_w", [1, C, 3, 3], F32, kind="ExternalInput").ap()
    w1_d = nc.dram_tensor("se_w1", [CMID, C, 3, 3], F32, kind="ExternalInput").ap()
    w2_d = nc.dram_tensor("se_w2", [1, CMID, 3, 3], F32, kind="ExternalInput").ap()
    out_d = nc.dram_tensor("out", [C, H, W], F32, kind="ExternalOutput").ap()

    x_v = x_d.rearrange("(b p) h w -> b p h w", b=2)                # [2,128,56,56]
    w_v = w_d.rearrange("(b p) ci kh kw -> b p (ci kh kw)", b=2)    # [2,128,2304]
    aw_v = aw_d[0].rearrange("(b p) kh kw -> b p (kh kw)", b=2)     # [2,128,9]
    w1_v = w1_d.rearrange("p ci kh kw -> p (ci kh kw)")             # [16,2304]
    w2_v = w2_d[0].rearrange("p kh kw -> p (kh kw)")                # [16,9]
    out_v = out_d.rearrange("(b p) h w -> b p h w", b=2)            # [2,128,56,56]

    with tile.TileContext(nc) as tc:
        with (
            tc.tile_pool(name="sb", bufs=1) as sb,
            tc.tile_pool(name="ps", space="PSUM", bufs=2) as ps,
        ):
            ident = sb.tile([128, 128], F32, name="ident")
            make_identity(nc, ident)

            # +2: one guard element at each flat end (dw=+-1 at image corners)
            xs = [sb.tile([128, HWP + 2], F32R, name=f"xs{i}") for i in range(2)]
            wr = [sb.tile([128, 2304], F32, name=f"wr{c}") for c in range(2)]
            aw = [sb.tile([128, 9], F32, name=f"aw{i}") for i in range(2)]
            w1s = sb.tile([CMID, 2304], F32, name="w1s")
            w2s = sb.tile([CMID, 9], F32, name="w2s")
            w2rep = sb.tile([CMID, 9 * 128], F32R, name="w2rep")
            wmod = [sb.tile([128, 2 * 9 * 128], F32R, name=f"wmod{i}") for i in range(2)]
            w1mod = [sb.tile([128, 9 * CMID], F32R, name=f"w1mod{i}") for i in range(2)]
            mid = sb.tile([CMID, HWP + 2], F32R, name="mid")
            asb = sb.tile([128, HWP], F32, name="asb")
            osb = [sb.tile([128, HWP], F32, name=f"osb{c}") for c in range(2)]

            # zero x/mid pads: guards + left/right pad columns. The interior
            # pad pairs (c=57 of row r, c=0 of row r+1) are flat-adjacent.
            for tl, np_ in ((xs[0], 128), (xs[1], 128), (mid, CMID)):
                tf = tl.bitcast(F32)
                nc.vector.memset(tf[:np_, 0:2], 0.0)                  # guard + c0 of row 0
                nc.vector.memset(tf[:np_, HWP : HWP + 2], 0.0)        # c57 of last row + guard
                pads = tf[:np_, 1 + W + 1 : 1 + W + 1 + (H - 1) * WP].rearrange(
                    "p (h c) -> p h c", c=WP
                )
                nc.vector.memset(pads[:, :, 0:2], 0.0)                # interior pad pairs

            # x into the padded interior
            for i in range(2):
                xsv = xs[i][:, 1 : 1 + HWP].rearrange("p (h c) -> p h c", c=WP)
                nc.sync.dma_start(xsv[:, :, 1 : W + 1], x_v[i].bitcast(F32R))
                nc.sync.dma_start(wr[i], w_v[i])
                nc.sync.dma_start(aw[i], aw_v[i])
            nc.sync.dma_start(w1s, w1_v)
            nc.sync.dma_start(w2s, w2_v)

            # w2 broadcast across conv2's 128 output partitions
            nc.vector.tensor_copy(
                w2rep.rearrange("p (k r) -> p k r", r=128),
                w2s.unsqueeze(2).broadcast_to([CMID, 9, 128]),
            )

            # main conv weights: transpose [co,ci] -> [ci,co] per tap, scale by A_w[ci,tap]
            for i in range(2):
                for c in range(2):
                    wrv = wr[c].rearrange("p (ci k) -> p ci k", k=9)
                    for k in range(9):
                        tp = ps.tile([128, 128], F32, name="tp", tag="tp", bufs=2)
                        nc.tensor.transpose(
                            tp, wrv[:, i * 128 : (i + 1) * 128, k], ident
                        )
                        nc.scalar.activation(
                            wmod[i][:, (c * 9 + k) * 128 : (c * 9 + k + 1) * 128],
                            tp,
                            mybir.ActivationFunctionType.Identity,
                            scale=aw[i][:, k : k + 1],
                        )

            # SE conv1 weights: [16,ci] -> [ci,16] per tap
            w1v = w1s.rearrange("p (ci k) -> p ci k", k=9)
            for i in range(2):
                for k in range(9):
                    tp1 = ps.tile([128, CMID], F32, name="tp1", tag="tp", bufs=2)
                    nc.tensor.transpose(
                        tp1, w1v[:, i * 128 : (i + 1) * 128, k], ident[:CMID, :CMID]
                    )
                    nc.vector.tensor_copy(
                        w1mod[i][:, k * CMID : (k + 1) * CMID], tp1
                    )

            mid_v = mid[:, 1 : 1 + HWP].rearrange("p (h c) -> p h c", c=WP)

            # ---- SE conv1: mid = relu(conv3x3(x, se_w1)) ----
            for t in range(NT):
                r0 = t * RT
                mps = ps.tile([CMID, RT * WP], F32, name="mps", tag="mid", bufs=2)
                n_mm = 0
                for dh, dw in TAPS:
                    k = (dh + 1) * 3 + (dw + 1)
                    rl, rh = _rows(r0, dh)
                    for i in range(2):
                        n_mm += 1
                        nc.tensor.matmul(
                            mps[:, rl * WP : rh * WP],
                            w1mod[i][:, k * CMID : (k + 1) * CMID],
                            xs[i][:, 1 + (r0 + rl + dh) * WP + dw :][: 128, : (rh - rl) * WP],
                            start=(n_mm == 1),
                            stop=(n_mm == 18),
                        )
                mpv = mps.rearrange("p (h c) -> p h c", c=WP)
                nc.scalar.activation(
                    mid_v[:, r0 : r0 + RT, 1 : W + 1],
                    mpv[:, :, 1 : W + 1],
                    mybir.ActivationFunctionType.Relu,
                )

            # ---- SE conv2 + sigmoid: a = sigmoid(conv3x3(mid, se_w2)) ----
            w2rep_v = w2rep.rearrange("p (k r) -> p k r", r=128)
            for t in range(NT):
                r0 = t * RT
                aps = ps.tile([128, RT * WP], F32, name="aps", tag="aps", bufs=2)
                n_mm = 0
                for dh, dw in TAPS:
                    k = (dh + 1) * 3 + (dw + 1)
                    rl, rh = _rows(r0, dh)
                    n_mm += 1
                    nc.tensor.matmul(
                        aps[:, rl * WP : rh * WP],
                        w2rep_v[:, k, :],
                        mid[:, 1 + (r0 + rl + dh) * WP + dw :][: CMID, : (rh - rl) * WP],
                        start=(n_mm == 1),
                        stop=(n_mm == 9),
                    )
                nc.scalar.activation(
                    asb[:, r0 * WP : (r0 + RT) * WP],
                    aps,
                    mybir.ActivationFunctionType.Sigmoid,
                )

            # ---- main conv + attention multiply ----
            for t in range(NT):
                r0 = t * RT
                for c in range(2):
                    yps = ps.tile([128, RT * WP], F32, name="yps", tag="yps", bufs=2)
                    n_mm = 0
                    for dh, dw in TAPS:
                        k = (dh + 1) * 3 + (dw + 1)
                        rl, rh = _rows(r0, dh)
                        for i in range(2):
                            n_mm += 1
                            nc.tensor.matmul(
                                yps[:, rl * WP : rh * WP],
                                wmod[i][:, (c * 9 + k) * 128 : (c * 9 + k + 1) * 128],
                                xs[i][:, 1 + (r0 + rl + dh) * WP + dw :][: 128, : (rh - rl) * WP],
                                start=(n_mm == 1),
                                stop=(n_mm == 18),
                            )
                    nc.vector.tensor_mul(
                        osb[c][:, r0 * WP : (r0 + RT) * WP],
                        yps,
                        asb[:, r0 * WP : (r0 + RT) * WP],
                    )
                    ov = osb[c].rearrange("p (h c) -> p h c", c=WP)
                    nc.sync.dma_start(
                        out_v[c][:, r0 : r0 + RT, :],
                        ov[:, r0 : r0 + RT, 1 : W + 1],
                    )

    nc.compile()
    return nc


_NC = None


def kernel(x, weight, A_w, se_w1, se_w2):
    global _NC
    if _NC is None:
        _NC = build()

    x = np.ascontiguousarray(np.asarray(x, dtype=np.float32))
    weight = np.ascontiguousarray(np.asarray(weight, dtype=np.float32))
    A_w = np.ascontiguousarray(np.asarray(A_w, dtype=np.float32))
    se_w1 = np.ascontiguousarray(np.asarray(se_w1, dtype=np.float32))
    se_w2 = np.ascontiguousarray(np.asarray(se_w2, dtype=np.float32))

    in_maps = [
        {
            "x": np.ascontiguousarray(x[b]),
            "weight": weight,
            "A_w": A_w,
            "se_w1": se_w1,
            "se_w2": se_w2,
        }
        for b in range(B)
    ]
    res = run_bass_kernel_spmd(_NC, in_maps, list(range(N_CORES)))
    out = np.stack([res.results[b]["out"] for b in range(B)], axis=0)
    return out


# revision 9
# speedup vs baseline: 1.0802x; 1.0802x over previous
"""Trainium2 Bass kernel for: out = conv3x3(x, weight*A_w) * sigmoid(conv3x3(relu(conv3x3(x, se_w1)), se_w2))

Sharding: data-parallel over batch B=8 -> 8 NeuronCores (one image per core);
weight / A_w / se_w1 / se_w2 replicated to every core.

Per-core kernel (direct conv as implicit GEMM on the TensorEngine):
  - x stored column-padded [ci, 56, 58] in SBUF (pad cols zeroed, +1-element
    guards at both flat ends) so every 3x3 tap is a contiguous 1-D shifted
    window (the matmul ISA requires single-free-dim operands).
  - row taps at the image top/bottom use clipped row ranges; with the center
    tap issued first (full coverage, start=True) the clipped taps accumulate,
    which implements zero padding exactly.
  - weights transposed on-device (PE transpose) to [ci, co] per tap; A_w is
    applied as a VectorE per-partition scale during the PSUM->SBUF drain.
  - compute dtype bf16 (accumulate fp32 in PSUM): full-rate matmuls with
    fast-weight-load, so the per-matmul LDWEIGHTS hides under the previous
    matmul. rel-err vs fp32 reference ~2e-3, inside the 2e-2 gate.
  - junk values only ever land in pad columns; they stay finite (sigmoid etc.)
    and are dropped by the strided output DMA.
"""

import numpy as np

import concourse.bass as bass  # noqa: F401
import concourse.mybir as mybir
import concourse.tile as tile
from concourse import bacc
from concourse.bass_utils import run_bass_kernel_spmd
from concourse.masks import make_identity

B, C, H, W = 8, 256, 56, 56
HW = H * W
WP = W + 2                      # padded row width (c=0 left pad, c=57 right pad)
HWP = H * WP                    # 3248
CMID = 16
N_CORES = 8
RT = 8                          # output rows per PSUM tile
NT = H // RT                    # 7
F32 = mybir.dt.float32
BF16 = mybir.dt.bfloat16

TAPS = [(0, 0)] + [
    (dh, dw) for dh in (-1, 0, 1) for dw in (-1, 0, 1) if (dh, dw) != (0, 0)
]


def _rows(r0, dh):
    """Clipped local row range [rl, rh) of a tile at base row r0 for row-tap dh."""
    return max(0, -dh - r0), min(RT, H - dh - r0)


def build():
    nc = bacc.Bacc("TRN2", target_bir_lowering=False, debug=False, num_devices=N_CORES)

    x_d = nc.dram_tensor("x", [C, H, W], F32, kind="ExternalInput").ap()
    w_d = nc.dram_tensor("weight", [C, C, 3, 3], F32, kind="ExternalInput").ap()
    aw_d = nc.dram_tensor("A_w", [1, C, 3, 3], F32, kind="ExternalInput").ap()
    w1_d = nc.dram_tensor("se_w1", [CMID, C, 3, 3], F32, kind="ExternalInput").ap()
    w2_d = nc.dram_tensor("se_w2", [1, CMID, 3, 3], F32, kind="ExternalInput").ap()
    out_d = nc.dram_tensor("out", [C, H, W], F32, kind="ExternalOutput").ap()

    x_v = x_d.rearrange("(b p) h w -> b p (h w)", b=2)              # [2,128,3136]
    w_v = w_d.rearrange("(b p) ci kh kw -> b p (ci kh kw)", b=2)    # [2,128,2304]
    aw_v = aw_d[0].rearrange("(b p) kh kw -> b p (kh kw)", b=2)     # [2,128,9]
    w1_v = w1_d.rearrange("p ci kh kw -> p (ci kh kw)")             # [16,2304]
    w2_v = w2_d[0].rearrange("p kh kw -> p (kh kw)")                # [16,9]
    out_v = out_d.rearrange("(b p) h w -> b p h w", b=2)            # [2,128,56,56]

    with tile.TileContext(nc) as tc:
        with (
            tc.tile_pool(name="sb", bufs=1) as sb,
            tc.tile_pool(name="ps", space="PSUM", bufs=2) as ps,
        ):
            ident = sb.tile([128, 128], F32, name="ident")
            make_identity(nc, ident)

            # +2: one guard element at each flat end (dw=+-1 at image corners)
            xs = [sb.tile([128, HWP + 2], BF16, name=f"xs{i}") for i in range(2)]
            xstage = [sb.tile([128, HW], F32, name=f"xstage{i}") for i in range(2)]
            wr = [sb.tile([128, 2304], F32, name=f"wr{c}") for c in range(2)]
            aw = [sb.tile([128, 9], F32, name=f"aw{i}") for i in range(2)]
            w1s = sb.tile([CMID, 2304], F32, name="w1s")
            w2s = sb.tile([CMID, 9], F32, name="w2s")
            w2rep = sb.tile([CMID, 9 * 128], BF16, name="w2rep")
            wmod = [sb.tile([128, 2 * 9 * 128], BF16, name=f"wmod{i}") for i in range(2)]
            w1mod = [sb.tile([128, 9 * CMID], BF16, name=f"w1mod{i}") for i in range(2)]
            mid = sb.tile([CMID, HWP + 2], BF16, name="mid")
            asb = sb.tile([128, HWP], F32, name="asb")
            osb = [sb.tile([128, HWP], F32, name=f"osb{c}") for c in range(2)]

            # weights first: the PE transposes depend on them (sync HWDGE queue)
            for c in range(2):
                nc.sync.dma_start(wr[c], w_v[c])
            # x on the scalar HWDGE queue so it streams in parallel with weights
            for i in range(2):
                nc.scalar.dma_start(xstage[i], x_v[i])
            # small params via SWDGE
            for i in range(2):
                nc.gpsimd.dma_start(aw[i], aw_v[i])
            nc.gpsimd.dma_start(w1s, w1_v)
            nc.gpsimd.dma_start(w2s, w2_v)

            # zero x/mid pads: guards + left/right pad columns. The interior
            # pad pairs (c=57 of row r, c=0 of row r+1) are flat-adjacent.
            for tl, np_ in ((xs[0], 128), (xs[1], 128), (mid, CMID)):
                nc.vector.memset(tl[:np_, 0:2], 0.0)                  # guard + c0 of row 0
                nc.vector.memset(tl[:np_, HWP : HWP + 2], 0.0)        # c57 of last row + guard
                pads = tl[:np_, 1 + W + 1 : 1 + W + 1 + (H - 1) * WP].rearrange(
                    "p (h c) -> p h c", c=WP
                )
                nc.vector.memset(pads[:, :, 0:2], 0.0)                # interior pad pairs

            # cast x into the padded bf16 interior
            for i in range(2):
                xsv = xs[i][:, 1 : 1 + HWP].rearrange("p (h c) -> p h c", c=WP)
                nc.vector.tensor_copy(
                    xsv[:, :, 1 : W + 1],
                    xstage[i].rearrange("p (h w) -> p h w", w=W),
                )

            # w2 broadcast across conv2's 128 output partitions
            nc.vector.tensor_copy(
                w2rep.rearrange("p (k r) -> p k r", r=128),
                w2s.unsqueeze(2).broadcast_to([CMID, 9, 128]),
            )

            # main conv weights: transpose [co,ci] -> [ci,co] per tap, scale by A_w[ci,tap]
            for i in range(2):
                for c in range(2):
                    wrv = wr[c].rearrange("p (ci k) -> p ci k", k=9)
                    for k in range(9):
                        tp = ps.tile([128, 128], F32, name="tp", tag="tp", bufs=2)
                        nc.tensor.transpose(
                            tp, wrv[:, i * 128 : (i + 1) * 128, k], ident
                        )
                        nc.vector.tensor_scalar_mul(
                            wmod[i][:, (c * 9 + k) * 128 : (c * 9 + k + 1) * 128],
                            tp,
                            aw[i][:, k : k + 1],
                        )

            # SE conv1 weights: [16,ci] -> [ci,16] per tap
            w1v = w1s.rearrange("p (ci k) -> p ci k", k=9)
            for i in range(2):
                for k in range(9):
                    tp1 = ps.tile([128, CMID], F32, name="tp1", tag="tp", bufs=2)
                    nc.tensor.transpose(
                        tp1, w1v[:, i * 128 : (i + 1) * 128, k], ident[:CMID, :CMID]
                    )
                    nc.vector.tensor_copy(
                        w1mod[i][:, k * CMID : (k + 1) * CMID], tp1
                    )

            mid_v = mid[:, 1 : 1 + HWP].rearrange("p (h c) -> p h c", c=WP)

            # ---- SE conv1: mid = relu(conv3x3(x, se_w1)) ----
            for t in range(NT):
                r0 = t * RT
                mps = ps.tile([CMID, RT * WP], F32, name="mps", tag="mid", bufs=2)
                n_mm = 0
                for dh, dw in TAPS:
                    k = (dh + 1) * 3 + (dw + 1)
                    rl, rh = _rows(r0, dh)
                    for i in range(2):
                        n_mm += 1
                        nc.tensor.matmul(
                            mps[:, rl * WP : rh * WP],
                            w1mod[i][:, k * CMID : (k + 1) * CMID],
                            xs[i][:, 1 + (r0 + rl + dh) * WP + dw :][: 128, : (rh - rl) * WP],
                            start=(n_mm == 1),
                            stop=(n_mm == 18),
                        )
                mpv = mps.rearrange("p (h c) -> p h c", c=WP)
                nc.scalar.activation(
                    mid_v[:, r0 : r0 + RT, 1 : W + 1],
                    mpv[:, :, 1 : W + 1],
                    mybir.ActivationFunctionType.Relu,
                )

            # ---- SE conv2 + sigmoid: a = sigmoid(conv3x3(mid, se_w2)) ----
            # lhsT columns replicate w2 across all 128 partitions so `a` lands
            # broadcast over the partition dim, ready for the final multiply.
            w2rep_v = w2rep.rearrange("p (k r) -> p k r", r=128)
            for t in range(NT):
                r0 = t * RT
                aps = ps.tile([128, RT * WP], F32, name="aps", tag="aps", bufs=2)
                n_mm = 0
                for dh, dw in TAPS:
                    k = (dh + 1) * 3 + (dw + 1)
                    rl, rh = _rows(r0, dh)
                    n_mm += 1
                    nc.tensor.matmul(
                        aps[:, rl * WP : rh * WP],
                        w2rep_v[:, k, :],
                        mid[:, 1 + (r0 + rl + dh) * WP + dw :][: CMID, : (rh - rl) * WP],
                        start=(n_mm == 1),
                        stop=(n_mm == 9),
                    )
                nc.scalar.activation(
                    asb[:, r0 * WP : (r0 + RT) * WP],
                    aps,
                    mybir.ActivationFunctionType.Sigmoid,
                )

            # ---- main conv + attention multiply ----
            for t in range(NT):
                r0 = t * RT
                for c in range(2):
                    yps = ps.tile([128, RT * WP], F32, name="yps", tag="yps", bufs=2)
                    n_mm = 0
                    for dh, dw in TAPS:
                        k = (dh + 1) * 3 + (dw + 1)
                        rl, rh = _rows(r0, dh)
                        for i in range(2):
                            n_mm += 1
                            nc.tensor.matmul(
                                yps[:, rl * WP : rh * WP],
                                wmod[i][:, (c * 9 + k) * 128 : (c * 9 + k + 1) * 128],
                                xs[i][:, 1 + (r0 + rl + dh) * WP + dw :][: 128, : (rh - rl) * WP],
                                start=(n_mm == 1),
                                stop=(n_mm == 18),
                            )
                    nc.vector.tensor_mul(
                        osb[c][:, r0 * WP : (r0 + RT) * WP],
                        yps,
                        asb[:, r0 * WP : (r0 + RT) * WP],
                    )
                    ov = osb[c].rearrange("p (h c) -> p h c", c=WP)
                    nc.sync.dma_start(
                        out_v[c][:, r0 : r0 + RT, :],
                        ov[:, r0 : r0 + RT, 1 : W + 1],
                    )

    nc.compile()
    return nc


_NC = None


def kernel(x, weight, A_w, se_w1, se_w2):
    global _NC
    if _NC is None:
        _NC = build()

    x = np.ascontiguousarray(np.asarray(x, dtype=np.float32))
    weight = np.ascontiguousarray(np.asarray(weight, dtype=np.float32))
    A_w = np.ascontiguousarray(np.asarray(A_w, dtype=np.float32))
    se_w1 = np.ascontiguousarray(np.asarray(se_w1, dtype=np.float32))
    se_w2 = np.ascontiguousarray(np.asarray(se_w2, dtype=np.float32))

    in_maps = [
        {
            "x": np.ascontiguousarray(x[b]),
            "weight": weight,
            "A_w": A_w,
            "se_w1": se_w1,
            "se_w2": se_w2,
        }
        for b in range(B)
    ]
    res = run_bass_kernel_spmd(_NC, in_maps, list(range(N_CORES)))
    out = np.stack([res.results[b]["out"] for b in range(B)], axis=0)
    return out


# revision 11
# speedup vs baseline: 1.3065x; 1.2095x over previous
"""Trainium2 Bass kernel for: out = conv3x3(x, weight*A_w) * sigmoid(conv3x3(relu(conv3x3(x, se_w1)), se_w2))

Sharding: data-parallel over batch B=8 -> 8 NeuronCores (one image per core);
weight / A_w / se_w1 / se_w2 replicated to every core. The conv weights are
passed transposed to [ci, kh, kw, co] (host-side layout prep during sharding)
so the matmul stationary operand loads straight from DRAM.

Per-core kernel (direct conv as implicit GEMM on the TensorEngine):
  - x stored column-padded [ci, 56, 58] bf16 in SBUF (pad cols zeroed,
    +1-element guards at both flat ends) so every 3x3 tap is a contiguous
    1-D shifted window (the matmul ISA requires single-free-dim operands).
  - row taps at the image top/bottom use clipped row ranges; the center tap
    is issued first per ci-block pass (full coverage, start=True), the
    clipped taps accumulate -> exact zero-padding semantics.
  - A_w applied on-device as a VectorE broadcast multiply during the
    f32 -> bf16 weight cast.
  - compute dtype bf16 (fp32 PSUM accumulate), rel-err vs fp32 ~3e-3.
  - thin SE-branch matmul groups (16-wide) are interleaved with dense
    128x128 main-conv groups to keep the PE activity monitor from
    re-throttling the clock (HAM).
  - main-conv PSUM tiles drain to SBUF; the attention multiply is fused
    when `a` for that tile is already available, otherwise applied in a
    deferred VectorE pass before the output DMA.
"""

import numpy as np

import concourse.bass as bass  # noqa: F401
import concourse.mybir as mybir
import concourse.tile as tile
from concourse import bacc
from concourse.bass_utils import run_bass_kernel_spmd

B, C, H, W = 8, 256, 56, 56
HW = H * W
WP = W + 2                      # padded row width (c=0 left pad, c=57 right pad)
HWP = H * WP                    # 3248
CMID = 16
N_CORES = 8
RT = 8                          # output rows per PSUM tile
NT = H // RT                    # 7
F32 = mybir.dt.float32
BF16 = mybir.dt.bfloat16

# center tap first within each ci-block pass
TAPS = [(0, 0)] + [
    (dh, dw) for dh in (-1, 0, 1) for dw in (-1, 0, 1) if (dh, dw) != (0, 0)
]


def _rows(r0, dh):
    """Clipped local row range [rl, rh) of a tile at base row r0 for row-tap dh."""
    return max(0, -dh - r0), min(RT, H - dh - r0)


def build():
    nc = bacc.Bacc("TRN2", target_bir_lowering=False, debug=False, num_devices=N_CORES)

    x_d = nc.dram_tensor("x", [C, H, W], F32, kind="ExternalInput").ap()
    # transposed on host: [ci, kh, kw, co]
    wt_d = nc.dram_tensor("weightT", [C, 3, 3, C], F32, kind="ExternalInput").ap()
    aw_d = nc.dram_tensor("A_w", [1, C, 3, 3], F32, kind="ExternalInput").ap()
    # transposed on host: [ci, kh, kw, cmid]
    w1t_d = nc.dram_tensor("se_w1T", [C, 3, 3, CMID], F32, kind="ExternalInput").ap()
    w2_d = nc.dram_tensor("se_w2", [1, CMID, 3, 3], F32, kind="ExternalInput").ap()
    out_d = nc.dram_tensor("out", [C, H, W], F32, kind="ExternalOutput").ap()

    x_v = x_d.rearrange("(b p) h w -> b p (h w)", b=2)                  # [2,128,3136]
    wt_v = wt_d.rearrange("(b p) kh kw co -> b p (kh kw co)", b=2)      # [2,128,2304]
    aw_v = aw_d[0].rearrange("(b p) kh kw -> b p (kh kw)", b=2)         # [2,128,9]
    w1t_v = w1t_d.rearrange("(b p) kh kw co -> b p (kh kw co)", b=2)    # [2,128,144]
    w2_v = w2_d[0].rearrange("p kh kw -> p (kh kw)")                    # [16,9]
    out_v = out_d.rearrange("(b p) h w -> b p h w", b=2)                # [2,128,56,56]

    with tile.TileContext(nc) as tc:
        with (
            tc.tile_pool(name="sb", bufs=1) as sb,
            tc.tile_pool(name="ps", space="PSUM", bufs=2) as ps,
        ):
            # +2: one guard element at each flat end (dw=+-1 at image corners)
            xs = [sb.tile([128, HWP + 2], BF16, name=f"xs{i}") for i in range(2)]
            xstage = [sb.tile([128, HW], F32, name=f"xstage{i}") for i in range(2)]
            wrt = [sb.tile([128, 2304], F32, name=f"wrt{i}") for i in range(2)]
            w1rt = [sb.tile([128, 9 * CMID], F32, name=f"w1rt{i}") for i in range(2)]
            aw = [sb.tile([128, 9], F32, name=f"aw{i}") for i in range(2)]
            w2s = sb.tile([CMID, 9], F32, name="w2s")
            w2rep = sb.tile([CMID, 9 * 128], BF16, name="w2rep")
            wmod = [sb.tile([128, 9 * 256], BF16, name=f"wmod{i}") for i in range(2)]
            w1mod = [sb.tile([128, 9 * CMID], BF16, name=f"w1mod{i}") for i in range(2)]
            mid = sb.tile([CMID, HWP + 2], BF16, name="mid")
            asb = sb.tile([128, HWP], F32, name="asb")
            osb = [sb.tile([128, HWP], F32, name=f"osb{c}") for c in range(2)]

            # -------- loads (weights on sync queue, x on scalar queue) --------
            for i in range(2):
                nc.sync.dma_start(wrt[i], wt_v[i])
                nc.scalar.dma_start(xstage[i], x_v[i])
            for i in range(2):
                nc.gpsimd.dma_start(w1rt[i], w1t_v[i])
                nc.gpsimd.dma_start(aw[i], aw_v[i])
            nc.gpsimd.dma_start(w2s, w2_v)

            # -------- weight prep (VectorE only, no PE) --------
            for i in range(2):
                # wmod[ci, k, co] = weightT[ci, k, co] * A_w[ci, k]  (cast to bf16)
                nc.vector.tensor_mul(
                    wmod[i].rearrange("p (k co) -> p k co", co=256),
                    wrt[i].rearrange("p (k co) -> p k co", co=256),
                    aw[i].unsqueeze(2).broadcast_to([128, 9, 256]),
                )
                nc.vector.tensor_copy(w1mod[i], w1rt[i])
            nc.vector.tensor_copy(
                w2rep.rearrange("p (k r) -> p k r", r=128),
                w2s.unsqueeze(2).broadcast_to([CMID, 9, 128]),
            )

            # -------- x pad + cast --------
            for tl, np_ in ((xs[0], 128), (xs[1], 128), (mid, CMID)):
                nc.vector.memset(tl[:np_, 0:2], 0.0)
                nc.vector.memset(tl[:np_, HWP : HWP + 2], 0.0)
                pads = tl[:np_, 1 + W + 1 : 1 + W + 1 + (H - 1) * WP].rearrange(
                    "p (h c) -> p h c", c=WP
                )
                nc.vector.memset(pads[:, :, 0:2], 0.0)
            for i in range(2):
                xsv = xs[i][:, 1 : 1 + HWP].rearrange("p (h c) -> p h c", c=WP)
                nc.vector.tensor_copy(
                    xsv[:, :, 1 : W + 1],
                    xstage[i].rearrange("p (h w) -> p h w", w=W),
                )

            mid_v = mid[:, 1 : 1 + HWP].rearrange("p (h c) -> p h c", c=WP)
            w2rep_v = w2rep.rearrange("p (k r) -> p k r", r=128)
            wmod_v = [wmod[i].rearrange("p (k co) -> p k co", co=256) for i in range(2)]

            # -------- conv group emitters --------
            def conv1_group(t):
                r0 = t * RT
                mps = ps.tile([CMID, RT * WP], F32, name="mps", tag="mid", bufs=2)
                n_mm = 0
                for i in range(2):
                    for dh, dw in TAPS:
                        k = (dh + 1) * 3 + (dw + 1)
                        rl, rh = _rows(r0, dh)
                        n_mm += 1
                        nc.tensor.matmul(
                            mps[:, rl * WP : rh * WP],
                            w1mod[i][:, k * CMID : (k + 1) * CMID],
                            xs[i][:, 1 + (r0 + rl + dh) * WP + dw :][:128, : (rh - rl) * WP],
                            start=(n_mm == 1),
                            stop=(n_mm == 18),
                        )
                mpv = mps.rearrange("p (h c) -> p h c", c=WP)
                nc.scalar.activation(
                    mid_v[:, r0 : r0 + RT, 1 : W + 1],
                    mpv[:, :, 1 : W + 1],
                    mybir.ActivationFunctionType.Relu,
                )

            def conv2_group(t):
                r0 = t * RT
                aps = ps.tile([128, RT * WP], F32, name="aps", tag="aps", bufs=2)
                n_mm = 0
                for dh, dw in TAPS:
                    k = (dh + 1) * 3 + (dw + 1)
                    rl, rh = _rows(r0, dh)
                    n_mm += 1
                    nc.tensor.matmul(
                        aps[:, rl * WP : rh * WP],
                        w2rep_v[:, k, :],
                        mid[:, 1 + (r0 + rl + dh) * WP + dw :][:CMID, : (rh - rl) * WP],
                        start=(n_mm == 1),
                        stop=(n_mm == 9),
                    )
                nc.scalar.activation(
                    asb[:, r0 * WP : (r0 + RT) * WP],
                    aps,
                    mybir.ActivationFunctionType.Sigmoid,
                )

            def main_group(t, c, fused):
                r0 = t * RT
                yps = ps.tile([128, RT * WP], F32, name="yps", tag="yps", bufs=4)
                n_mm = 0
                for i in range(2):
                    for dh, dw in TAPS:
                        k = (dh + 1) * 3 + (dw + 1)
                        rl, rh = _rows(r0, dh)
                        n_mm += 1
                        nc.tensor.matmul(
                            yps[:, rl * WP : rh * WP],
                            wmod_v[i][:, k, c * 128 : (c + 1) * 128],
                            xs[i][:, 1 + (r0 + rl + dh) * WP + dw :][:128, : (rh - rl) * WP],
                            start=(n_mm == 1),
                            stop=(n_mm == 18),
                        )
                dst = osb[c][:, r0 * WP : (r0 + RT) * WP]
                if fused:
                    nc.vector.tensor_mul(dst, yps, asb[:, r0 * WP : (r0 + RT) * WP])
                    ov = osb[c].rearrange("p (h c) -> p h c", c=WP)
                    nc.sync.dma_start(
                        out_v[c][:, r0 : r0 + RT, :], ov[:, r0 : r0 + RT, 1 : W + 1]
                    )
                else:
                    nc.vector.tensor_copy(dst, yps)

            # -------- interleaved schedule --------
            # main groups in issue order; SE groups threaded between them so
            # the PE never sees a long run of thin (16-wide) matmuls.
            main_q = [(t, c) for t in range(NT) for c in range(2)]
            mq = iter(main_q)
            deferred = []
            sig_done = [False] * NT

            def emit_main(n, fused_allowed):
                for _ in range(n):
                    tc_ = next(mq, None)
                    if tc_ is None:
                        return
                    t, c = tc_
                    if sig_done[t] and fused_allowed:
                        main_group(t, c, fused=True)
                    else:
                        main_group(t, c, fused=False)
                        deferred.append((t, c))

            for t in range(NT):
                conv1_group(t)
                emit_main(1, fused_allowed=False)
            for t in range(NT):
                conv2_group(t)
                sig_done[t] = True
                emit_main(1, fused_allowed=True)
            # remaining main groups: `a` is fully available, fuse the multiply
            emit_main(len(main_q), fused_allowed=True)

            # deferred attention multiplies + output DMA
            for t, c in deferred:
                r0 = t * RT
                dst = osb[c][:, r0 * WP : (r0 + RT) * WP]
                nc.vector.tensor_mul(dst, dst, asb[:, r0 * WP : (r0 + RT) * WP])
                ov = osb[c].rearrange("p (h c) -> p h c", c=WP)
                nc.sync.dma_start(
                    out_v[c][:, r0 : r0 + RT, :], ov[:, r0 : r0 + RT, 1 : W + 1]
                )

    nc.compile()
    return nc


_NC = None


def make_in_maps(x, weight, A_w, se_w1, se_w2):
    x = np.ascontiguousarray(np.asarray(x, dtype=np.float32))
    weightT = np.ascontiguousarray(
        np.asarray(weight, dtype=np.float32).transpose(1, 2, 3, 0)
    )
    A_w = np.ascontiguousarray(np.asarray(A_w, dtype=np.float32))
    se_w1T = np.ascontiguousarray(
        np.asarray(se_w1, dtype=np.float32).transpose(1, 2, 3, 0)
    )
    se_w2 = np.ascontiguousarray(np.asarray(se_w2, dtype=np.float32))

    in_maps = [
        {
            "x": np.ascontiguousarray(x[b]),
            "weightT": weightT,
            "A_w": A_w,
            "se_w1T": se_w1T,
            "se_w2": se_w2,
        }
        for b in range(B)
    ]
    return in_maps


def kernel(x, weight, A_w, se_w1, se_w2):
    global _NC
    if _NC is None:
        _NC = build()
    in_maps = make_in_maps(x, weight, A_w, se_w1, se_w2)
    res = run_bass_kernel_spmd(_NC, in_maps, list(range(N_CORES)))
    out = np.stack([res.results[b]["out"] for b in range(B)], axis=0)
    return out


# revision 12
# speedup vs baseline: 1.4199x; 1.0868x over previous
"""Trainium2 Bass kernel for: out = conv3x3(x, weight*A_w) * sigmoid(conv3x3(relu(conv3x3(x, se_w1)), se_w2))

Sharding: data-parallel over batch B=8 -> 8 NeuronCores (one image per core);
weight / A_w / se_w1 / se_w2 replicated to every core. The conv weights are
passed transposed to [ci, kh, kw, co] (host-side layout prep during sharding)
so the matmul stationary operand loads straight from DRAM.

Per-core kernel (direct conv as implicit GEMM on the TensorEngine):
  - x stored column-padded [ci, 56, 58] bf16 in SBUF (pad cols zeroed,
    +1-element guards at both flat ends) so every 3x3 tap is a contiguous
    1-D shifted window (the matmul ISA requires single-free-dim operands).
  - row taps at the image top/bottom use clipped row ranges; the center tap
    is issued first per ci-block pass (full coverage, start=True), the
    clipped taps accumulate -> exact zero-padding semantics.
  - A_w applied on-device as a VectorE broadcast multiply during the
    f32 -> bf16 weight cast.
  - compute dtype bf16 (fp32 PSUM accumulate), rel-err vs fp32 ~3e-3.
  - thin SE-branch matmul groups (16-wide) are interleaved with dense
    128x128 main-conv groups to keep the PE activity monitor from
    re-throttling the clock (HAM).
  - main-conv PSUM tiles drain to SBUF; the attention multiply is fused
    when `a` for that tile is already available, otherwise applied in a
    deferred VectorE pass before the output DMA.
"""

import numpy as np

import concourse.bass as bass  # noqa: F401
import concourse.mybir as mybir
import concourse.tile as tile
from concourse import bacc
from concourse.bass_utils import run_bass_kernel_spmd

B, C, H, W = 8, 256, 56, 56
HW = H * W
WP = W + 2                      # padded row width (c=0 left pad, c=57 right pad)
HWP = H * WP                    # 3248
CMID = 16
N_CORES = 8
RT = 8                          # output rows per PSUM tile
NT = H // RT                    # 7
F32 = mybir.dt.float32
BF16 = mybir.dt.bfloat16

# center tap first within each ci-block pass
TAPS = [(0, 0)] + [
    (dh, dw) for dh in (-1, 0, 1) for dw in (-1, 0, 1) if (dh, dw) != (0, 0)
]


def _rows(r0, dh):
    """Clipped local row range [rl, rh) of a tile at base row r0 for row-tap dh."""
    return max(0, -dh - r0), min(RT, H - dh - r0)


def build():
    nc = bacc.Bacc("TRN2", target_bir_lowering=False, debug=False, num_devices=N_CORES)

    x_d = nc.dram_tensor("x", [C, H, W], F32, kind="ExternalInput").ap()
    # transposed on host: [ci, kh, kw, co]
    wt_d = nc.dram_tensor("weightT", [C, 3, 3, C], F32, kind="ExternalInput").ap()
    aw_d = nc.dram_tensor("A_w", [1, C, 3, 3], F32, kind="ExternalInput").ap()
    # transposed on host: [ci, kh, kw, cmid]
    w1t_d = nc.dram_tensor("se_w1T", [C, 3, 3, CMID], F32, kind="ExternalInput").ap()
    w2_d = nc.dram_tensor("se_w2", [1, CMID, 3, 3], F32, kind="ExternalInput").ap()
    out_d = nc.dram_tensor("out", [C, H, W], F32, kind="ExternalOutput").ap()

    x_v = x_d.rearrange("(b p) h w -> b p (h w)", b=2)                  # [2,128,3136]
    wt_v = wt_d.rearrange("(b p) kh kw co -> b p (kh kw co)", b=2)      # [2,128,2304]
    aw_v = aw_d[0].rearrange("(b p) kh kw -> b p (kh kw)", b=2)         # [2,128,9]
    w1t_v = w1t_d.rearrange("(b p) kh kw co -> b p (kh kw co)", b=2)    # [2,128,144]
    w2_v = w2_d[0].rearrange("p kh kw -> p (kh kw)")                    # [16,9]
    out_v = out_d.rearrange("(b p) h w -> b p h w", b=2)                # [2,128,56,56]

    with tile.TileContext(nc) as tc:
        with (
            tc.tile_pool(name="sb", bufs=1) as sb,
            tc.tile_pool(name="ps", space="PSUM", bufs=2) as ps,
        ):
            # +2: one guard element at each flat end (dw=+-1 at image corners)
            xs = [sb.tile([128, HWP + 2], BF16, name=f"xs{i}") for i in range(2)]
            xstage = [sb.tile([128, HW], F32, name=f"xstage{i}") for i in range(2)]
            wrt = [sb.tile([128, 2304], F32, name=f"wrt{i}") for i in range(2)]
            w1rt = [sb.tile([128, 9 * CMID], F32, name=f"w1rt{i}") for i in range(2)]
            aw = [sb.tile([128, 9], F32, name=f"aw{i}") for i in range(2)]
            w2s = sb.tile([CMID, 9], F32, name="w2s")
            w2rep = sb.tile([CMID, 9 * 128], BF16, name="w2rep")
            wmod = [sb.tile([128, 9 * 256], BF16, name=f"wmod{i}") for i in range(2)]
            w1mod = [sb.tile([128, 9 * CMID], BF16, name=f"w1mod{i}") for i in range(2)]
            mid = sb.tile([CMID, HWP + 2], BF16, name="mid")
            asb = sb.tile([128, HWP], F32, name="asb")
            osb = [sb.tile([128, HWP], F32, name=f"osb{c}") for c in range(2)]

            # -------- loads --------
            # x first (the PE's first dependency), chunked so the bf16 cast
            # pipelines behind the DMA; one ci-block per HWDGE queue.
            HHALF = H // 2
            for i in range(2):
                q = nc.scalar if i == 0 else nc.sync
                for h0 in (0, HHALF):
                    q.dma_start(
                        xstage[i][:, h0 * W : (h0 + HHALF) * W],
                        x_v[i][:, h0 * W : (h0 + HHALF) * W],
                    )
            for i in range(2):
                nc.sync.dma_start(wrt[i], wt_v[i])
            for i in range(2):
                nc.gpsimd.dma_start(w1rt[i], w1t_v[i])
                nc.gpsimd.dma_start(aw[i], aw_v[i])
            nc.gpsimd.dma_start(w2s, w2_v)

            # -------- x pad + cast (DVE, ahead of weight prep) --------
            for tl, np_ in ((xs[0], 128), (xs[1], 128), (mid, CMID)):
                nc.vector.memset(tl[:np_, 0:2], 0.0)
                nc.vector.memset(tl[:np_, HWP : HWP + 2], 0.0)
                pads = tl[:np_, 1 + W + 1 : 1 + W + 1 + (H - 1) * WP].rearrange(
                    "p (h c) -> p h c", c=WP
                )
                nc.vector.memset(pads[:, :, 0:2], 0.0)
            for h0 in (0, HHALF):
                for i in range(2):
                    xsv = xs[i][:, 1 : 1 + HWP].rearrange("p (h c) -> p h c", c=WP)
                    nc.vector.tensor_copy(
                        xsv[:, h0 : h0 + HHALF, 1 : W + 1],
                        xstage[i][:, h0 * W : (h0 + HHALF) * W].rearrange(
                            "p (h w) -> p h w", w=W
                        ),
                    )

            # -------- weight prep (VectorE only, no PE) --------
            for i in range(2):
                # wmod[ci, k, co] = weightT[ci, k, co] * A_w[ci, k]  (cast to bf16)
                nc.vector.tensor_mul(
                    wmod[i].rearrange("p (k co) -> p k co", co=256),
                    wrt[i].rearrange("p (k co) -> p k co", co=256),
                    aw[i].unsqueeze(2).broadcast_to([128, 9, 256]),
                )
                nc.vector.tensor_copy(w1mod[i], w1rt[i])
            nc.vector.tensor_copy(
                w2rep.rearrange("p (k r) -> p k r", r=128),
                w2s.unsqueeze(2).broadcast_to([CMID, 9, 128]),
            )

            mid_v = mid[:, 1 : 1 + HWP].rearrange("p (h c) -> p h c", c=WP)
            w2rep_v = w2rep.rearrange("p (k r) -> p k r", r=128)
            wmod_v = [wmod[i].rearrange("p (k co) -> p k co", co=256) for i in range(2)]

            # -------- conv group emitters --------
            def conv1_group(t):
                r0 = t * RT
                mps = ps.tile([CMID, RT * WP], F32, name="mps", tag="mid", bufs=2)
                n_mm = 0
                for i in range(2):
                    for dh, dw in TAPS:
                        k = (dh + 1) * 3 + (dw + 1)
                        rl, rh = _rows(r0, dh)
                        n_mm += 1
                        nc.tensor.matmul(
                            mps[:, rl * WP : rh * WP],
                            w1mod[i][:, k * CMID : (k + 1) * CMID],
                            xs[i][:, 1 + (r0 + rl + dh) * WP + dw :][:128, : (rh - rl) * WP],
                            start=(n_mm == 1),
                            stop=(n_mm == 18),
                        )
                mpv = mps.rearrange("p (h c) -> p h c", c=WP)
                nc.scalar.activation(
                    mid_v[:, r0 : r0 + RT, 1 : W + 1],
                    mpv[:, :, 1 : W + 1],
                    mybir.ActivationFunctionType.Relu,
                )

            def conv2_group(t):
                r0 = t * RT
                aps = ps.tile([128, RT * WP], F32, name="aps", tag="aps", bufs=2)
                n_mm = 0
                for dh, dw in TAPS:
                    k = (dh + 1) * 3 + (dw + 1)
                    rl, rh = _rows(r0, dh)
                    n_mm += 1
                    nc.tensor.matmul(
                        aps[:, rl * WP : rh * WP],
                        w2rep_v[:, k, :],
                        mid[:, 1 + (r0 + rl + dh) * WP + dw :][:CMID, : (rh - rl) * WP],
                        start=(n_mm == 1),
                        stop=(n_mm == 9),
                    )
                nc.scalar.activation(
                    asb[:, r0 * WP : (r0 + RT) * WP],
                    aps,
                    mybir.ActivationFunctionType.Sigmoid,
                )

            def main_group(t, c, fused):
                r0 = t * RT
                yps = ps.tile([128, RT * WP], F32, name="yps", tag="yps", bufs=4)
                n_mm = 0
                for i in range(2):
                    for dh, dw in TAPS:
                        k = (dh + 1) * 3 + (dw + 1)
                        rl, rh = _rows(r0, dh)
                        n_mm += 1
                        nc.tensor.matmul(
                            yps[:, rl * WP : rh * WP],
                            wmod_v[i][:, k, c * 128 : (c + 1) * 128],
                            xs[i][:, 1 + (r0 + rl + dh) * WP + dw :][:128, : (rh - rl) * WP],
                            start=(n_mm == 1),
                            stop=(n_mm == 18),
                        )
                dst = osb[c][:, r0 * WP : (r0 + RT) * WP]
                if fused:
                    nc.vector.tensor_mul(dst, yps, asb[:, r0 * WP : (r0 + RT) * WP])
                    ov = osb[c].rearrange("p (h c) -> p h c", c=WP)
                    nc.sync.dma_start(
                        out_v[c][:, r0 : r0 + RT, :], ov[:, r0 : r0 + RT, 1 : W + 1]
                    )
                else:
                    nc.vector.tensor_copy(dst, yps)

            # -------- interleaved schedule --------
            # main groups in issue order; SE groups threaded between them so
            # the PE never sees a long run of thin (16-wide) matmuls.
            main_q = [(t, c) for t in range(NT) for c in range(2)]
            mq = iter(main_q)
            deferred = []
            sig_done = [False] * NT

            def emit_main(n, fused_allowed):
                for _ in range(n):
                    tc_ = next(mq, None)
                    if tc_ is None:
                        return
                    t, c = tc_
                    if sig_done[t] and fused_allowed:
                        main_group(t, c, fused=True)
                    else:
                        main_group(t, c, fused=False)
                        deferred.append((t, c))

            def flush_deferred():
                rest = []
                for t, c in deferred:
                    if not sig_done[t]:
                        rest.append((t, c))
                        continue
                    r0 = t * RT
                    dst = osb[c][:, r0 * WP : (r0 + RT) * WP]
                    nc.vector.tensor_mul(dst, dst, asb[:, r0 * WP : (r0 + RT) * WP])
                    ov = osb[c].rearrange("p (h c) -> p h c", c=WP)
                    nc.sync.dma_start(
                        out_v[c][:, r0 : r0 + RT, :], ov[:, r0 : r0 + RT, 1 : W + 1]
                    )
                deferred[:] = rest

            for t in range(NT):
                conv1_group(t)
                emit_main(1, fused_allowed=False)
            for t in range(NT):
                conv2_group(t)
                sig_done[t] = True
                emit_main(1, fused_allowed=True)
                flush_deferred()
            # remaining main groups: `a` is fully available, fuse the multiply
            emit_main(len(main_q), fused_allowed=True)
            flush_deferred()

    nc.compile()
    return nc


_NC = None


def make_in_maps(x, weight, A_w, se_w1, se_w2):
    x = np.ascontiguousarray(np.asarray(x, dtype=np.float32))
    weightT = np.ascontiguousarray(
        np.asarray(weight, dtype=np.float32).transpose(1, 2, 3, 0)
    )
    A_w = np.ascontiguousarray(np.asarray(A_w, dtype=np.float32))
    se_w1T = np.ascontiguousarray(
        np.asarray(se_w1, dtype=np.float32).transpose(1, 2, 3, 0)
    )
    se_w2 = np.ascontiguousarray(np.asarray(se_w2, dtype=np.float32))

    in_maps = [
        {
            "x": np.ascontiguousarray(x[b]),
            "weightT": weightT,
            "A_w": A_w,
            "se_w1T": se_w1T,
            "se_w2": se_w2,
        }
        for b in range(B)
    ]
    return in_maps


def kernel(x, weight, A_w, se_w1, se_w2):
    global _NC
    if _NC is None:
        _NC = build()
    in_maps = make_in_maps(x, weight, A_w, se_w1, se_w2)
    res = run_bass_kernel_spmd(_NC, in_maps, list(range(N_CORES)))
    out = np.stack([res.results[b]["out"] for b in range(B)], axis=0)
    return out
